# revision 46
# baseline (speedup 1.0000x reference)
"""GQA attention (S=2048, D=4096, 32 Q heads / 8 KV heads, RoPE, full attn)
distributed over 8 Trainium2 NeuronCores.

Strategy (tensor-parallel by heads, Megatron-style with AllGathers before
the output projection instead of an AllReduce after it):
  - core c owns Q heads 4c..4c+3 and KV head c (GQA groups align with cores).
  - projections computed as transposed GEMMs: QT/KT [chan, tok] directly
    usable by the scores matmul; V via VT + PE transposes.
  - RoPE folded into two PE "mix" matmuls over host-deinterleaved channels,
    scale folded into wq on the host.
  - all big matmuls in bf16 (enables Fast Weight Load; fp32/f32r stationary
    operands serialize a ~200ns LDWEIGHTS per matmul), f32 PSUM accumulate.
  - scores transposed, ST = KT.T @ QT -> [k, q]; exp on ScalarE (bf16 out);
    softmax normalizer: bf16 DVE partial sums -> ones-matmul -> ln/exp on
    ScalarE -> gpsimd partition_broadcast -> DVE scale. Entire chain off-PE.
  - per-(chunk, head-pair) AllGathers (2MB each) so gathering starts
    mid-attention; gathered channel order is hp-major (wo columns permuted
    on the host to match).
  - the wo GEMM of chunk qc is INTERLEAVED into the attention kt-loop of
    chunk qc+1 (4 matmuls per kt slot starting at slot 4): the PE queue
    never idles while the scalar engine streams exps.

Host side only reshapes/transposes/pads/casts and concatenates outputs.
"""
import sys

import numpy as np
import ml_dtypes

_BF16 = ml_dtypes.bfloat16

for _p in ("/root/.axon_site/_ro/trn_rl_repo", "/opt/trn_rl_repo"):
    if _p not in sys.path:
        sys.path.append(_p)

import concourse.bass as bass
import concourse.tile as tile
from concourse import mybir
from concourse.bass_utils import run_bass_kernel_spmd

N_CORES = 8
S = 2048
D = 4096
HD = 128
N_QH = 4          # Q heads per core
N_KT = S // 128   # 16 k-tiles
N_TC = S // 512   # 4 token chunks
N_KC = D // 128   # 32 contraction tiles
F32 = mybir.dt.float32
F32R = mybir.dt.float32r
BF16 = mybir.dt.bfloat16

_NC_CACHE = {}


def _split_multi_waits(nc):
    """This container's walrus accepts only ONE sync-wait per instruction
    encoding; hoist extra waits onto fresh single-wait NoOps placed before
    the instruction on the same engine."""
    n = 0
    for fn in nc.m.functions:
        for bb in fn.blocks:
            new_insts = []
            changed = False
            for ins in bb.instructions:
                si = ins.sync_info
                waits = list(si.on_wait) if si is not None else []
                if len(waits) > 1:
                    for w in waits[:-1]:
                        n += 1
                        nop = mybir.InstNoOp(name=f"WSPL-{n}", ins=[], outs=[])
                        nop.engine = ins.engine
                        nop.sync_info = mybir.SyncInfo(on_wait=[w], on_update=[])
                        new_insts.append(nop)
                    si.on_wait = waits[-1:]
                    changed = True
                new_insts.append(ins)
            if changed:
                bb.instructions = new_insts
    return n


def _build():
    nc = bass.Bass()

    xt = nc.dram_tensor("xt", [D, S], BF16, kind="ExternalInput")
    wqt = nc.dram_tensor("wqt", [D, 512], BF16, kind="ExternalInput")
    wkt = nc.dram_tensor("wkt", [D, HD], BF16, kind="ExternalInput")
    wvt = nc.dram_tensor("wvt", [D, HD], BF16, kind="ExternalInput")
    wot = nc.dram_tensor("wot", [D, 512], BF16, kind="ExternalInput")
    cs1 = nc.dram_tensor("cs1", [HD, S], F32, kind="ExternalInput")
    cs2 = nc.dram_tensor("cs2", [HD, S], F32, kind="ExternalInput")
    mix1 = nc.dram_tensor("mix1", [HD, HD], BF16, kind="ExternalInput")
    mix2 = nc.dram_tensor("mix2", [HD, HD], BF16, kind="ExternalInput")
    onesc = nc.dram_tensor("onesc", [HD, 1], BF16, kind="ExternalInput")
    onesr = nc.dram_tensor("onesr", [1, HD], F32R, kind="ExternalInput")
    ident = nc.dram_tensor("ident", [HD, HD], BF16, kind="ExternalInput")
    out_ext = nc.dram_tensor("out", [S, 512], F32, kind="ExternalOutput")

    # one AllGather per q-chunk: attention outpaces the CC stream's ~13us
    # per-op overhead, so fewer, larger collectives win.
    ag_in = [
        nc.dram_tensor(f"agi{qc}", [512, 512], BF16) for qc in range(N_TC)
    ]
    ag_out = [
        nc.dram_tensor(f"ago{qc}", [D, 512], BF16, addr_space="Shared")
        for qc in range(N_TC)
    ]

    xt_r = xt.rearrange("(kc p) s -> kc p s", p=128)
    wqt_r = wqt.rearrange("(kc p) n -> kc p n", p=128)
    wkt_r = wkt.rearrange("(kc p) n -> kc p n", p=128)
    wvt_r = wvt.rearrange("(kc p) n -> kc p n", p=128)
    wot_r = wot.rearrange("(hk p) n -> hk p n", p=128)

    with tile.TileContext(nc) as tc:
        with (
            tc.tile_pool(name="const", bufs=1) as constp,
            tc.tile_pool(name="persist", bufs=1) as persist,
        ):
            cs1_sb = constp.tile([HD, S], F32)
            cs2_sb = constp.tile([HD, S], F32)
            mix1_sb = constp.tile([HD, HD], BF16)
            mix2_sb = constp.tile([HD, HD], BF16)
            onesc_sb = constp.tile([HD, 1], BF16)
            onesr_sb = constp.tile([1, HD], F32R)
            ident_sb = constp.tile([HD, HD], BF16)

            qt_sb = persist.tile([128, N_QH, S], BF16)   # QT_rope
            kt_sb = persist.tile([128, S], BF16)         # KT_rope
            v_sb = persist.tile([128, N_KT, HD], BF16)   # V [tok-in-tile, kt, chan]

            # ---------------- phase 1: projections + rope ----------------
            with (
                tc.tile_pool(name="wq", bufs=1) as wqp,
                tc.tile_pool(name="xtp", bufs=3) as xtp,
                tc.tile_pool(name="uv", bufs=2) as uvp,
                tc.tile_pool(name="vt", bufs=2) as vtp,
                tc.tile_pool(name="p1q", bufs=1, space="PSUM") as p1q,
                tc.tile_pool(name="p1k", bufs=1, space="PSUM") as p1k,
                tc.tile_pool(name="p1r", bufs=1, space="PSUM") as p1r,
            ):
                wq_sb = wqp.tile([128, N_KC, 512], BF16)
                wk_sb = wqp.tile([128, N_KC, HD], BF16)
                wv_sb = wqp.tile([128, N_KC, HD], BF16)

                nc.gpsimd.dma_start(
                    out=wq_sb[:, 0:4, :],
                    in_=wqt_r[0:4].rearrange("kc p n -> p kc n"),
                )
                nc.gpsimd.dma_start(out=wk_sb[:], in_=wkt_r[:].rearrange("kc p n -> p kc n"))
                nc.gpsimd.dma_start(out=wv_sb[:], in_=wvt_r[:].rearrange("kc p n -> p kc n"))
                nc.gpsimd.dma_start(
                    out=wq_sb[:, 4:8, :],
                    in_=wqt_r[4:8].rearrange("kc p n -> p kc n"),
                )
                for ch in (1, 2, 3):
                    nc.gpsimd.dma_start(
                        out=wq_sb[:, ch * 8:(ch + 1) * 8, :],
                        in_=wqt_r[ch * 8:(ch + 1) * 8].rearrange("kc p n -> p kc n"),
                    )
                nc.gpsimd.dma_start(out=cs1_sb[:], in_=cs1[:])
                nc.gpsimd.dma_start(out=cs2_sb[:], in_=cs2[:])
                nc.gpsimd.dma_start(out=mix1_sb[:], in_=mix1[:])
                nc.gpsimd.dma_start(out=mix2_sb[:], in_=mix2[:])
                nc.gpsimd.dma_start(out=onesc_sb[:], in_=onesc[:])
                nc.gpsimd.dma_start(out=onesr_sb[:], in_=onesr[:])
                nc.gpsimd.dma_start(out=ident_sb[:], in_=ident[:])

                for tcb in range(N_TC):
                    t0 = tcb * 512
                    scope = nc.named_scope(f"proj{tcb}"); scope.__enter__()
                    qps = [
                        p1q.tile([128, 512], F32, name=f"qps{tcb}_{h}", tag=f"qps{h}")
                        for h in range(N_QH)
                    ]
                    kps = p1k.tile([128, 512], F32, name=f"kps{tcb}", tag="kps")
                    vtps = p1k.tile([128, 512], F32, name=f"vtps{tcb}", tag="vtps")
                    xt_g = None
                    for kc in range(N_KC):
                        if kc % 8 == 0:
                            xt_g = xtp.tile([128, 8, 512], BF16, name=f"xt{tcb}_{kc}", tag="xt")
                            nc.sync.dma_start(
                                out=xt_g[:],
                                in_=xt_r[kc:kc + 8, :, t0:t0 + 512].rearrange("g p n -> p g n"),
                            )
                        xt_t = xt_g[:, kc % 8, :]
                        st, sp = kc == 0, kc == N_KC - 1
                        for h in range(N_QH):
                            nc.tensor.matmul(
                                qps[h][:], wq_sb[:, kc, h * 128:(h + 1) * 128],
                                xt_t, start=st, stop=sp,
                            )
                        nc.tensor.matmul(kps[:], wk_sb[:, kc, :], xt_t, start=st, stop=sp)
                        nc.tensor.matmul(vtps[:], wv_sb[:, kc, :], xt_t, start=st, stop=sp)

                    # rope: K first (attention depends on full KT), then Q heads
                    for h in [N_QH] + list(range(N_QH)):
                        src = kps if h == N_QH else qps[h]
                        u_t = uvp.tile([128, 512], BF16, name=f"u{tcb}_{h}", tag="u")
                        v_t = uvp.tile([128, 512], BF16, name=f"v{tcb}_{h}", tag="v")
                        nc.vector.tensor_mul(u_t[:], src[:], cs1_sb[:, t0:t0 + 512])
                        nc.vector.tensor_mul(v_t[:], src[:], cs2_sb[:, t0:t0 + 512])
                        rps = p1r.tile([128, 512], F32, name=f"rps{tcb}_{h}", tag="rps")
                        nc.tensor.matmul(rps[:], mix1_sb[:], u_t[:], start=True, stop=False)
                        nc.tensor.matmul(rps[:], mix2_sb[:], v_t[:], start=False, stop=True)
                        if h == N_QH:
                            nc.vector.tensor_copy(kt_sb[:, t0:t0 + 512], rps[:])
                        else:
                            nc.vector.tensor_copy(qt_sb[:, h, t0:t0 + 512], rps[:])

                    # V for this token chunk: VT -> PE transpose -> V
                    vt_sb = vtp.tile([128, 512], BF16, name=f"vts{tcb}", tag="vts")
                    nc.vector.tensor_copy(vt_sb[:], vtps[:])
                    vtr = p1r.tile([128, 4, 128], BF16, name=f"vtr{tcb}", tag="vtr")
                    for j in range(4):
                        nc.tensor.transpose(
                            vtr[:, j, :], vt_sb[:, j * 128:(j + 1) * 128],
                            ident_sb[:],
                        )
                    nc.vector.tensor_copy(v_sb[:, tcb * 4:(tcb + 1) * 4, :], vtr[:])
                    scope.__exit__(None, None, None)

            # ---- phase 2: attention, with prev chunk's wo GEMM woven in ----
            with (
                tc.tile_pool(name="wo", bufs=1) as wop,
                tc.tile_pool(name="ep", bufs=4) as ep,
                tc.tile_pool(name="zp", bufs=1) as zp,
                tc.tile_pool(name="np_", bufs=2) as np_,
                tc.tile_pool(name="agp", bufs=8) as agp,
                tc.tile_pool(name="fout", bufs=2) as foutp,
                tc.tile_pool(name="p2s", bufs=2, space="PSUM") as p2s,
                tc.tile_pool(name="p2pv", bufs=1, space="PSUM") as p2pv,
                tc.tile_pool(name="p4f", bufs=1, space="PSUM") as p4f,
            ):
                wo_sb = wop.tile([128, N_KC, 512], BF16)
                nc.gpsimd.dma_start(out=wo_sb[:], in_=wot_r[:].rearrange("hk p n -> p hk n"))

                def attn_chunk(qc, fill=None):
                    # fill: chunk index whose wo matmuls (first 2 token
                    # subtiles, m=0..63) are woven into hp1's kt loop — its
                    # gather is staged well before hp1 starts, so the filler
                    # never stalls the attention stream.
                    q0 = qc * 512
                    scope = nc.named_scope(f"attn{qc}"); scope.__enter__()
                    fill_n = [0]

                    def fill_mms(upto):
                        if fill is None:
                            return
                        while fill_n[0] < min(upto, 64):
                            wo_mm(fill, fill_n[0])
                            fill_n[0] += 1

                    for hp in range(2):
                        hs = [2 * hp, 2 * hp + 1]
                        pvs = {
                            h: p2pv.tile([128, 512], F32, name=f"pv{qc}_{h}", tag=f"pv{h % 2}")
                            for h in hs
                        }
                        zparts = {
                            h: zp.tile([128, 512], BF16, name=f"zpt{qc}_{h}", tag=f"zpart{h % 2}")
                            for h in hs
                        }
                        for kt in range(N_KT):
                            k0 = kt * 128
                            stp = p2s.tile([128, 2, 512], F32, name=f"st{qc}_{hp}_{kt}", tag="st")
                            for j, h in enumerate(hs):
                                nc.tensor.matmul(
                                    stp[:, j, :], kt_sb[:, k0:k0 + 128],
                                    qt_sb[:, h, q0:q0 + 512], start=True, stop=True,
                                )
                            e_t = ep.tile([128, 2, 512], BF16, name=f"e{qc}_{kt}_{hp}", tag="e")
                            nc.scalar.activation(
                                out=e_t[:], in_=stp[:],
                                func=mybir.ActivationFunctionType.Exp,
                            )
                            for j, h in enumerate(hs):
                                if kt == 0:
                                    nc.vector.tensor_copy(zparts[h][:], e_t[:, j, :])
                                else:
                                    nc.vector.tensor_add(zparts[h][:], zparts[h][:], e_t[:, j, :])
                            for j, h in enumerate(hs):
                                nc.tensor.matmul(
                                    pvs[h][:], v_sb[:, kt, :], e_t[:, j, :],
                                    start=(kt == 0), stop=(kt == N_KT - 1),
                                )
                            if hp == 1:
                                fill_mms((kt + 1) * 4)
                        zpss = {}
                        for h in hs:
                            zpss[h] = p2s.tile([1, 512], F32, name=f"zps{qc}_{h}", tag="st")
                            nc.tensor.matmul(zpss[h][:], onesc_sb[:], zparts[h][:], start=True, stop=True)
                        invzs = {}
                        for h in hs:
                            lnz = zp.tile([1, 512], F32, name=f"lnz{qc}_{h}", tag=f"lnz{h % 2}")
                            nc.scalar.activation(
                                out=lnz[:], in_=zpss[h][:],
                                func=mybir.ActivationFunctionType.Ln,
                            )
                            invzs[h] = zp.tile([1, 512], F32R, name=f"izr{qc}_{h}", tag=f"invz{h % 2}")
                            nc.scalar.activation(
                                out=invzs[h][:], in_=lnz[:],
                                func=mybir.ActivationFunctionType.Exp, scale=-1.0,
                            )
                        for h in hs:
                            bcps = p2s.tile([128, 512], F32, name=f"bc{qc}_{h}", tag="st")
                            nc.tensor.matmul(bcps[:], onesr_sb[:], invzs[h][:], start=True, stop=True)
                            bc_sb = np_.tile([128, 512], F32, name=f"bcs{qc}_{h}", tag="bcs")
                            nc.vector.tensor_copy(bc_sb[:], bcps[:])
                            at_sb = np_.tile([128, 512], BF16, name=f"at{qc}_{h}", tag="at")
                            nc.vector.tensor_mul(at_sb[:], pvs[h][:], bc_sb[:])
                            nc.sync.dma_start(
                                out=ag_in[qc][h * 128:(h + 1) * 128, :],
                                in_=at_sb[:],
                            )
                    sc2 = nc.named_scope(f"ag{qc}"); sc2.__enter__()
                    nc.gpsimd.collective_compute(
                        "AllGather",
                        mybir.AluOpType.bypass,
                        replica_groups=[list(range(N_CORES))],
                        ins=[ag_in[qc][:].opt()],
                        outs=[ag_out[qc][:].opt()],
                    )
                    sc2.__exit__(None, None, None)
                    # queue the SBUF staging loads NOW: behind only this
                    # chunk's trigger, never behind later chunks' triggers
                    # (which block on collective completion).
                    ago_r = ag_out[qc].rearrange("(hk p) n -> hk p n", p=128)
                    rhs[qc] = []
                    for g in range(4):
                        rhs_g = agp.tile([128, 8, 512], BF16, name=f"ag{qc}_{g}", tag="ag")
                        nc.gpsimd.dma_start(
                            out=rhs_g[:],
                            in_=ago_r[g * 8:(g + 1) * 8].rearrange("g p n -> p g n"),
                        )
                        rhs[qc].append(rhs_g)
                    scope.__exit__(None, None, None)

                rhs = {}
                wstate = {}

                def wo_mm(wq_c, m):
                    # one wo matmul, m = qs*32 + hk; qs-sequential with
                    # alternating PSUM banks
                    qs, hk = m // 32, m % 32
                    if hk == 0:
                        wstate[wq_c] = p4f.tile(
                            [128, 512], F32, name=f"f{wq_c}_{qs}", tag=f"f{qs % 2}"
                        )
                    fp = wstate[wq_c]
                    rhs_gs = rhs[wq_c]
                    nc.tensor.matmul(
                        fp[:],
                        rhs_gs[hk // 8][:, hk % 8, qs * 128:(qs + 1) * 128],
                        wo_sb[:, hk, :], start=(hk == 0), stop=(hk == N_KC - 1),
                    )
                    if hk == N_KC - 1:
                        f_sb = foutp.tile([128, 512], F32, name=f"fs{wq_c}_{qs}", tag="fs")
                        nc.vector.tensor_copy(f_sb[:], fp[:])
                        nc.sync.dma_start(
                            out=out_ext[wq_c * 512 + qs * 128:wq_c * 512 + (qs + 1) * 128, :],
                            in_=f_sb[:],
                        )

                def wo_rest(qc, m_from):
                    scope = nc.named_scope(f"wo{qc}"); scope.__enter__()
                    for m in range(m_from, 128):
                        wo_mm(qc, m)
                    scope.__exit__(None, None, None)

                # attention first (the scalar exp stream paces it and gates
                # each chunk's AllGather); the first half of wo(qc-2) fills
                # the PE's scalar-wait slivers in attn(qc)'s second head
                # pair, by which point its gather is staged; the remainder
                # runs after attn3.
                attn_chunk(0)
                attn_chunk(1)
                attn_chunk(2, fill=0)
                attn_chunk(3, fill=1)
                wo_rest(0, 64)
                wo_rest(1, 64)
                wo_rest(2, 0)
                wo_rest(3, 0)

    _split_multi_waits(nc)
    return nc


def _host_prep(x, cos, sin, wq, wk, wv, wo):
    scale = np.float32(HD ** -0.5)
    perm = np.concatenate([np.arange(0, HD, 2), np.arange(1, HD, 2)])

    xT = np.ascontiguousarray(x.T.astype(_BF16))
    cosT = np.ascontiguousarray(cos.T)
    sinT = np.ascontiguousarray(sin.T)
    cs1 = np.concatenate([cosT, sinT], axis=0)
    cs2 = np.concatenate([sinT, cosT], axis=0)

    m1 = np.zeros((HD, HD), np.float32)
    m1[np.arange(64), np.arange(64)] = 1.0
    m1[np.arange(64) + 64, np.arange(64)] = -1.0
    m2 = np.zeros((HD, HD), np.float32)
    m2[np.arange(64), np.arange(64) + 64] = 1.0
    m2[np.arange(64) + 64, np.arange(64) + 64] = 1.0

    # gathered attn channel order: for hp in (0,1): for core c: heads
    # (4c+2hp, 4c+2hp+1). Permute wo's input columns to match.
    shared = {
        "xt": xT,
        "cs1": cs1,
        "cs2": cs2,
        "mix1": m1.astype(_BF16),
        "mix2": m2.astype(_BF16),
        "onesc": np.ones((HD, 1), np.float32).astype(_BF16),
        "onesr": np.ones((1, HD), np.float32),
        "ident": np.eye(HD, dtype=np.float32).astype(_BF16),
    }
    in_maps = []
    for c in range(N_CORES):
        wq_c = wq[c * 512:(c + 1) * 512].reshape(N_QH, HD, D)[:, perm, :]
        wq_c = (wq_c.reshape(512, D) * scale)
        wk_c = wk[c * HD:(c + 1) * HD][perm, :]
        wv_c = wv[c * HD:(c + 1) * HD]
        wo_c = wo[c * 512:(c + 1) * 512]
        m = dict(shared)
        m["wqt"] = np.ascontiguousarray(wq_c.T).astype(_BF16)
        m["wkt"] = np.ascontiguousarray(wk_c.T).astype(_BF16)
        m["wvt"] = np.ascontiguousarray(wv_c.T).astype(_BF16)
        m["wot"] = np.ascontiguousarray(wo_c.T).astype(_BF16)
        in_maps.append(m)
    return in_maps


def kernel(x, cos, sin, wq, wk, wv, wo, _trace=False):
    x = np.asarray(x, np.float32)
    cos = np.asarray(cos, np.float32)
    sin = np.asarray(sin, np.float32)
    wq = np.asarray(wq, np.float32)
    wk = np.asarray(wk, np.float32)
    wv = np.asarray(wv, np.float32)
    wo = np.asarray(wo, np.float32)

    in_maps = _host_prep(x, cos, sin, wq, wk, wv, wo)
    if "nc" not in _NC_CACHE:
        _NC_CACHE["nc"] = _build()
    nc = _NC_CACHE["nc"]
    res = run_bass_kernel_spmd(
        nc, in_maps, core_ids=list(range(N_CORES)), trace=_trace
    )
    out = np.concatenate([res.results[c]["out"] for c in range(N_CORES)], axis=1)
    out = np.ascontiguousarray(out, dtype=np.float32)
    if _trace:
        kernel._last_exec_time_ns = res.exec_time_ns
        kernel._last_result = res
    return out


# revision 47
# speedup vs baseline: 1.0430x; 1.0430x over previous
"""GQA attention (S=2048, D=4096, 32 Q heads / 8 KV heads, RoPE, full attn)
distributed over 8 Trainium2 NeuronCores.

Strategy (tensor-parallel by heads, Megatron-style with AllGathers before
the output projection instead of an AllReduce after it):
  - core c owns Q heads 4c..4c+3 and KV head c (GQA groups align with cores).
  - projections computed as transposed GEMMs: QT/KT [chan, tok] directly
    usable by the scores matmul; V via VT + PE transposes.
  - RoPE folded into two PE "mix" matmuls over host-deinterleaved channels,
    scale folded into wq on the host.
  - all big matmuls in bf16 (enables Fast Weight Load; fp32/f32r stationary
    operands serialize a ~200ns LDWEIGHTS per matmul), f32 PSUM accumulate.
  - scores transposed, ST = KT.T @ QT -> [k, q]; exp on ScalarE (bf16 out);
    softmax normalizer: bf16 DVE partial sums -> ones-matmul -> ln/exp on
    ScalarE -> gpsimd partition_broadcast -> DVE scale. Entire chain off-PE.
  - per-(chunk, head-pair) AllGathers (2MB each) so gathering starts
    mid-attention; gathered channel order is hp-major (wo columns permuted
    on the host to match).
  - the wo GEMM of chunk qc is INTERLEAVED into the attention kt-loop of
    chunk qc+1 (4 matmuls per kt slot starting at slot 4): the PE queue
    never idles while the scalar engine streams exps.

Host side only reshapes/transposes/pads/casts and concatenates outputs.
"""
import sys

import numpy as np
import ml_dtypes

_BF16 = ml_dtypes.bfloat16

for _p in ("/root/.axon_site/_ro/trn_rl_repo", "/opt/trn_rl_repo"):
    if _p not in sys.path:
        sys.path.append(_p)

import concourse.bass as bass
import concourse.tile as tile
from concourse import mybir
from concourse.bass_utils import run_bass_kernel_spmd

N_CORES = 8
S = 2048
D = 4096
HD = 128
N_QH = 4          # Q heads per core
N_KT = S // 128   # 16 k-tiles
N_TC = S // 512   # 4 token chunks
N_KC = D // 128   # 32 contraction tiles
F32 = mybir.dt.float32
F32R = mybir.dt.float32r
BF16 = mybir.dt.bfloat16

_NC_CACHE = {}


def _split_multi_waits(nc):
    """This container's walrus accepts only ONE sync-wait per instruction
    encoding; hoist extra waits onto fresh single-wait NoOps placed before
    the instruction on the same engine."""
    n = 0
    for fn in nc.m.functions:
        for bb in fn.blocks:
            new_insts = []
            changed = False
            for ins in bb.instructions:
                si = ins.sync_info
                waits = list(si.on_wait) if si is not None else []
                if len(waits) > 1:
                    for w in waits[:-1]:
                        n += 1
                        nop = mybir.InstNoOp(name=f"WSPL-{n}", ins=[], outs=[])
                        nop.engine = ins.engine
                        nop.sync_info = mybir.SyncInfo(on_wait=[w], on_update=[])
                        new_insts.append(nop)
                    si.on_wait = waits[-1:]
                    changed = True
                new_insts.append(ins)
            if changed:
                bb.instructions = new_insts
    return n


def _build():
    nc = bass.Bass()

    xt = nc.dram_tensor("xt", [D, S], BF16, kind="ExternalInput")
    wqt = nc.dram_tensor("wqt", [D, 512], BF16, kind="ExternalInput")
    wkt = nc.dram_tensor("wkt", [D, HD], BF16, kind="ExternalInput")
    wvt = nc.dram_tensor("wvt", [D, HD], BF16, kind="ExternalInput")
    wot = nc.dram_tensor("wot", [D, 512], BF16, kind="ExternalInput")
    cs1 = nc.dram_tensor("cs1", [HD, S], F32, kind="ExternalInput")
    cs2 = nc.dram_tensor("cs2", [HD, S], F32, kind="ExternalInput")
    mix1 = nc.dram_tensor("mix1", [HD, HD], BF16, kind="ExternalInput")
    mix2 = nc.dram_tensor("mix2", [HD, HD], BF16, kind="ExternalInput")
    onesc = nc.dram_tensor("onesc", [HD, 1], BF16, kind="ExternalInput")
    onesr = nc.dram_tensor("onesr", [1, HD], F32R, kind="ExternalInput")
    ident = nc.dram_tensor("ident", [HD, HD], BF16, kind="ExternalInput")
    out_ext = nc.dram_tensor("out", [S, 512], F32, kind="ExternalOutput")

    # one AllGather per q-chunk: attention outpaces the CC stream's ~13us
    # per-op overhead, so fewer, larger collectives win.
    ag_in = [
        nc.dram_tensor(f"agi{qc}", [512, 512], BF16) for qc in range(N_TC)
    ]
    ag_out = [
        nc.dram_tensor(f"ago{qc}", [D, 512], BF16, addr_space="Shared")
        for qc in range(N_TC)
    ]

    xt_r = xt.rearrange("(kc p) s -> kc p s", p=128)
    wqt_r = wqt.rearrange("(kc p) n -> kc p n", p=128)
    wkt_r = wkt.rearrange("(kc p) n -> kc p n", p=128)
    wvt_r = wvt.rearrange("(kc p) n -> kc p n", p=128)
    wot_r = wot.rearrange("(hk p) n -> hk p n", p=128)

    with tile.TileContext(nc) as tc:
        with (
            tc.tile_pool(name="const", bufs=1) as constp,
            tc.tile_pool(name="persist", bufs=1) as persist,
        ):
            cs1_sb = constp.tile([HD, S], F32)
            cs2_sb = constp.tile([HD, S], F32)
            mix1_sb = constp.tile([HD, HD], BF16)
            mix2_sb = constp.tile([HD, HD], BF16)
            onesc_sb = constp.tile([HD, 1], BF16)
            onesr_sb = constp.tile([1, HD], F32R)
            ident_sb = constp.tile([HD, HD], BF16)

            qt_sb = persist.tile([128, N_QH, S], BF16)   # QT_rope
            kt_sb = persist.tile([128, S], BF16)         # KT_rope
            v_sb = persist.tile([128, N_KT, HD], BF16)   # V [tok-in-tile, kt, chan]

            # ---------------- phase 1: projections + rope ----------------
            with (
                tc.tile_pool(name="wq", bufs=1) as wqp,
                tc.tile_pool(name="xtp", bufs=3) as xtp,
                tc.tile_pool(name="uv", bufs=2) as uvp,
                tc.tile_pool(name="vt", bufs=2) as vtp,
                tc.tile_pool(name="p1q", bufs=1, space="PSUM") as p1q,
                tc.tile_pool(name="p1k", bufs=1, space="PSUM") as p1k,
                tc.tile_pool(name="p1r", bufs=1, space="PSUM") as p1r,
            ):
                wq_sb = wqp.tile([128, N_KC, 512], BF16)
                wk_sb = wqp.tile([128, N_KC, HD], BF16)
                wv_sb = wqp.tile([128, N_KC, HD], BF16)

                nc.gpsimd.dma_start(
                    out=wq_sb[:, 0:4, :],
                    in_=wqt_r[0:4].rearrange("kc p n -> p kc n"),
                )
                nc.gpsimd.dma_start(out=wk_sb[:], in_=wkt_r[:].rearrange("kc p n -> p kc n"))
                nc.gpsimd.dma_start(out=wv_sb[:], in_=wvt_r[:].rearrange("kc p n -> p kc n"))
                nc.gpsimd.dma_start(
                    out=wq_sb[:, 4:8, :],
                    in_=wqt_r[4:8].rearrange("kc p n -> p kc n"),
                )
                for ch in (1, 2, 3):
                    nc.gpsimd.dma_start(
                        out=wq_sb[:, ch * 8:(ch + 1) * 8, :],
                        in_=wqt_r[ch * 8:(ch + 1) * 8].rearrange("kc p n -> p kc n"),
                    )
                nc.gpsimd.dma_start(out=cs1_sb[:], in_=cs1[:])
                nc.gpsimd.dma_start(out=cs2_sb[:], in_=cs2[:])
                nc.gpsimd.dma_start(out=mix1_sb[:], in_=mix1[:])
                nc.gpsimd.dma_start(out=mix2_sb[:], in_=mix2[:])
                nc.gpsimd.dma_start(out=onesc_sb[:], in_=onesc[:])
                nc.gpsimd.dma_start(out=onesr_sb[:], in_=onesr[:])
                nc.gpsimd.dma_start(out=ident_sb[:], in_=ident[:])

                for tcb in range(N_TC):
                    t0 = tcb * 512
                    scope = nc.named_scope(f"proj{tcb}"); scope.__enter__()
                    qps = [
                        p1q.tile([128, 512], F32, name=f"qps{tcb}_{h}", tag=f"qps{h}")
                        for h in range(N_QH)
                    ]
                    kps = p1k.tile([128, 512], F32, name=f"kps{tcb}", tag="kps")
                    vtps = p1k.tile([128, 512], F32, name=f"vtps{tcb}", tag="vtps")
                    xt_g = None
                    for kc in range(N_KC):
                        if kc % 8 == 0:
                            xt_g = xtp.tile([128, 8, 512], BF16, name=f"xt{tcb}_{kc}", tag="xt")
                            nc.sync.dma_start(
                                out=xt_g[:],
                                in_=xt_r[kc:kc + 8, :, t0:t0 + 512].rearrange("g p n -> p g n"),
                            )
                        xt_t = xt_g[:, kc % 8, :]
                        st, sp = kc == 0, kc == N_KC - 1
                        for h in range(N_QH):
                            nc.tensor.matmul(
                                qps[h][:], wq_sb[:, kc, h * 128:(h + 1) * 128],
                                xt_t, start=st, stop=sp,
                            )
                        nc.tensor.matmul(kps[:], wk_sb[:, kc, :], xt_t, start=st, stop=sp)
                        nc.tensor.matmul(vtps[:], wv_sb[:, kc, :], xt_t, start=st, stop=sp)

                    # rope: K first (attention depends on full KT), then Q heads
                    for h in [N_QH] + list(range(N_QH)):
                        src = kps if h == N_QH else qps[h]
                        u_t = uvp.tile([128, 512], BF16, name=f"u{tcb}_{h}", tag="u")
                        v_t = uvp.tile([128, 512], BF16, name=f"v{tcb}_{h}", tag="v")
                        nc.vector.tensor_mul(u_t[:], src[:], cs1_sb[:, t0:t0 + 512])
                        nc.vector.tensor_mul(v_t[:], src[:], cs2_sb[:, t0:t0 + 512])
                        rps = p1r.tile([128, 512], F32, name=f"rps{tcb}_{h}", tag="rps")
                        nc.tensor.matmul(rps[:], mix1_sb[:], u_t[:], start=True, stop=False)
                        nc.tensor.matmul(rps[:], mix2_sb[:], v_t[:], start=False, stop=True)
                        if h == N_QH:
                            nc.vector.tensor_copy(kt_sb[:, t0:t0 + 512], rps[:])
                        else:
                            nc.vector.tensor_copy(qt_sb[:, h, t0:t0 + 512], rps[:])

                    # V for this token chunk: VT -> PE transpose -> V
                    vt_sb = vtp.tile([128, 512], BF16, name=f"vts{tcb}", tag="vts")
                    nc.vector.tensor_copy(vt_sb[:], vtps[:])
                    vtr = p1r.tile([128, 4, 128], BF16, name=f"vtr{tcb}", tag="vtr")
                    for j in range(4):
                        nc.tensor.transpose(
                            vtr[:, j, :], vt_sb[:, j * 128:(j + 1) * 128],
                            ident_sb[:],
                        )
                    nc.vector.tensor_copy(v_sb[:, tcb * 4:(tcb + 1) * 4, :], vtr[:])
                    scope.__exit__(None, None, None)

            # ---- phase 2: attention, with prev chunk's wo GEMM woven in ----
            with (
                tc.tile_pool(name="wo", bufs=1) as wop,
                tc.tile_pool(name="ep", bufs=4) as ep,
                tc.tile_pool(name="zp", bufs=1) as zp,
                tc.tile_pool(name="np_", bufs=2) as np_,
                tc.tile_pool(name="agp", bufs=8) as agp,
                tc.tile_pool(name="fout", bufs=2) as foutp,
                tc.tile_pool(name="p2s", bufs=2, space="PSUM") as p2s,
                tc.tile_pool(name="p2pv", bufs=1, space="PSUM") as p2pv,
                tc.tile_pool(name="p4f", bufs=1, space="PSUM") as p4f,
            ):
                wo_sb = wop.tile([128, N_KC, 512], BF16)
                nc.gpsimd.dma_start(out=wo_sb[:], in_=wot_r[:].rearrange("hk p n -> p hk n"))

                def attn_chunk(qc, fill=None):
                    # fill: chunk index whose wo matmuls (first 2 token
                    # subtiles, m=0..63) are woven into hp1's kt loop — its
                    # gather is staged well before hp1 starts, so the filler
                    # never stalls the attention stream.
                    q0 = qc * 512
                    scope = nc.named_scope(f"attn{qc}"); scope.__enter__()
                    fill_n = [0]

                    def fill_mms(upto):
                        if fill is None:
                            return
                        while fill_n[0] < min(upto, 64):
                            wo_mm(fill, fill_n[0])
                            fill_n[0] += 1

                    for hp in range(2):
                        hs = [2 * hp, 2 * hp + 1]
                        pvs = {
                            h: p2pv.tile([128, 512], F32, name=f"pv{qc}_{h}", tag=f"pv{h % 2}")
                            for h in hs
                        }
                        zparts = {
                            h: zp.tile([128, 512], BF16, name=f"zpt{qc}_{h}", tag=f"zpart{h % 2}")
                            for h in hs
                        }
                        for kt in range(N_KT):
                            k0 = kt * 128
                            stp = p2s.tile([128, 2, 512], F32, name=f"st{qc}_{hp}_{kt}", tag="st")
                            for j, h in enumerate(hs):
                                nc.tensor.matmul(
                                    stp[:, j, :], kt_sb[:, k0:k0 + 128],
                                    qt_sb[:, h, q0:q0 + 512], start=True, stop=True,
                                )
                            e_t = ep.tile([128, 2, 512], BF16, name=f"e{qc}_{kt}_{hp}", tag="e")
                            nc.scalar.activation(
                                out=e_t[:], in_=stp[:],
                                func=mybir.ActivationFunctionType.Exp,
                            )
                            for j, h in enumerate(hs):
                                if kt == 0:
                                    nc.vector.tensor_copy(zparts[h][:], e_t[:, j, :])
                                else:
                                    nc.vector.tensor_add(zparts[h][:], zparts[h][:], e_t[:, j, :])
                            for j, h in enumerate(hs):
                                nc.tensor.matmul(
                                    pvs[h][:], v_sb[:, kt, :], e_t[:, j, :],
                                    start=(kt == 0), stop=(kt == N_KT - 1),
                                )
                            if hp == 1:
                                fill_mms((kt + 1) * 4)
                        zpss = {}
                        for h in hs:
                            zpss[h] = p2s.tile([1, 512], F32, name=f"zps{qc}_{h}", tag="st")
                            nc.tensor.matmul(zpss[h][:], onesc_sb[:], zparts[h][:], start=True, stop=True)
                        invzs = {}
                        for h in hs:
                            lnz = zp.tile([1, 512], F32, name=f"lnz{qc}_{h}", tag=f"lnz{h % 2}")
                            nc.scalar.activation(
                                out=lnz[:], in_=zpss[h][:],
                                func=mybir.ActivationFunctionType.Ln,
                            )
                            invzs[h] = zp.tile([1, 512], F32R, name=f"izr{qc}_{h}", tag=f"invz{h % 2}")
                            nc.scalar.activation(
                                out=invzs[h][:], in_=lnz[:],
                                func=mybir.ActivationFunctionType.Exp, scale=-1.0,
                            )
                        for h in hs:
                            bcps = p2s.tile([128, 512], F32, name=f"bc{qc}_{h}", tag="st")
                            nc.tensor.matmul(bcps[:], onesr_sb[:], invzs[h][:], start=True, stop=True)
                            bc_sb = np_.tile([128, 512], F32, name=f"bcs{qc}_{h}", tag="bcs")
                            nc.vector.tensor_copy(bc_sb[:], bcps[:])
                            at_sb = np_.tile([128, 512], BF16, name=f"at{qc}_{h}", tag="at")
                            nc.vector.tensor_mul(at_sb[:], pvs[h][:], bc_sb[:])
                            nc.sync.dma_start(
                                out=ag_in[qc][h * 128:(h + 1) * 128, :],
                                in_=at_sb[:],
                            )
                    sc2 = nc.named_scope(f"ag{qc}"); sc2.__enter__()
                    nc.gpsimd.collective_compute(
                        "AllGather",
                        mybir.AluOpType.bypass,
                        replica_groups=[list(range(N_CORES))],
                        ins=[ag_in[qc][:].opt()],
                        outs=[ag_out[qc][:].opt()],
                    )
                    sc2.__exit__(None, None, None)
                    # queue the SBUF staging loads NOW: behind only this
                    # chunk's trigger, never behind later chunks' triggers
                    # (which block on collective completion).
                    ago_r = ag_out[qc].rearrange("(hk p) n -> hk p n", p=128)
                    rhs[qc] = []
                    for g in range(4):
                        rhs_g = agp.tile([128, 8, 512], BF16, name=f"ag{qc}_{g}", tag="ag")
                        nc.gpsimd.dma_start(
                            out=rhs_g[:],
                            in_=ago_r[g * 8:(g + 1) * 8].rearrange("g p n -> p g n"),
                        )
                        rhs[qc].append(rhs_g)
                    scope.__exit__(None, None, None)

                rhs = {}
                wstate = {}

                def wo_mm(wq_c, m):
                    # one wo matmul, m = qs*32 + hk; qs-sequential with
                    # alternating PSUM banks
                    qs, hk = m // 32, m % 32
                    if hk == 0:
                        wstate[wq_c] = p4f.tile(
                            [128, 512], F32, name=f"f{wq_c}_{qs}", tag=f"f{qs % 2}"
                        )
                    fp = wstate[wq_c]
                    rhs_gs = rhs[wq_c]
                    nc.tensor.matmul(
                        fp[:],
                        rhs_gs[hk // 8][:, hk % 8, qs * 128:(qs + 1) * 128],
                        wo_sb[:, hk, :], start=(hk == 0), stop=(hk == N_KC - 1),
                    )
                    if hk == N_KC - 1:
                        f_sb = foutp.tile([128, 512], F32, name=f"fs{wq_c}_{qs}", tag="fs")
                        nc.vector.tensor_copy(f_sb[:], fp[:])
                        nc.sync.dma_start(
                            out=out_ext[wq_c * 512 + qs * 128:wq_c * 512 + (qs + 1) * 128, :],
                            in_=f_sb[:],
                        )

                def wo_rest(qc, m_from):
                    scope = nc.named_scope(f"wo{qc}"); scope.__enter__()
                    for m in range(m_from, 128):
                        wo_mm(qc, m)
                    scope.__exit__(None, None, None)

                # wo(qc) two chunks behind attn(qc): the saturated scalar exp
                # stream gates each chunk's AllGather, so earlier wo
                # placement stalls the in-order PE queue on ungathered data.
                attn_chunk(0)
                attn_chunk(1)
                attn_chunk(2)
                wo_rest(0, 0)
                attn_chunk(3)
                wo_rest(1, 0)
                wo_rest(2, 0)
                wo_rest(3, 0)

    _split_multi_waits(nc)
    return nc


def _host_prep(x, cos, sin, wq, wk, wv, wo):
    scale = np.float32(HD ** -0.5)
    perm = np.concatenate([np.arange(0, HD, 2), np.arange(1, HD, 2)])

    xT = np.ascontiguousarray(x.T.astype(_BF16))
    cosT = np.ascontiguousarray(cos.T)
    sinT = np.ascontiguousarray(sin.T)
    cs1 = np.concatenate([cosT, sinT], axis=0)
    cs2 = np.concatenate([sinT, cosT], axis=0)

    m1 = np.zeros((HD, HD), np.float32)
    m1[np.arange(64), np.arange(64)] = 1.0
    m1[np.arange(64) + 64, np.arange(64)] = -1.0
    m2 = np.zeros((HD, HD), np.float32)
    m2[np.arange(64), np.arange(64) + 64] = 1.0
    m2[np.arange(64) + 64, np.arange(64) + 64] = 1.0

    # gathered attn channel order: for hp in (0,1): for core c: heads
    # (4c+2hp, 4c+2hp+1). Permute wo's input columns to match.
    shared = {
        "xt": xT,
        "cs1": cs1,
        "cs2": cs2,
        "mix1": m1.astype(_BF16),
        "mix2": m2.astype(_BF16),
        "onesc": np.ones((HD, 1), np.float32).astype(_BF16),
        "onesr": np.ones((1, HD), np.float32),
        "ident": np.eye(HD, dtype=np.float32).astype(_BF16),
    }
    in_maps = []
    for c in range(N_CORES):
        wq_c = wq[c * 512:(c + 1) * 512].reshape(N_QH, HD, D)[:, perm, :]
        wq_c = (wq_c.reshape(512, D) * scale)
        wk_c = wk[c * HD:(c + 1) * HD][perm, :]
        wv_c = wv[c * HD:(c + 1) * HD]
        wo_c = wo[c * 512:(c + 1) * 512]
        m = dict(shared)
        m["wqt"] = np.ascontiguousarray(wq_c.T).astype(_BF16)
        m["wkt"] = np.ascontiguousarray(wk_c.T).astype(_BF16)
        m["wvt"] = np.ascontiguousarray(wv_c.T).astype(_BF16)
        m["wot"] = np.ascontiguousarray(wo_c.T).astype(_BF16)
        in_maps.append(m)
    return in_maps


def kernel(x, cos, sin, wq, wk, wv, wo, _trace=False):
    x = np.asarray(x, np.float32)
    cos = np.asarray(cos, np.float32)
    sin = np.asarray(sin, np.float32)
    wq = np.asarray(wq, np.float32)
    wk = np.asarray(wk, np.float32)
    wv = np.asarray(wv, np.float32)
    wo = np.asarray(wo, np.float32)

    in_maps = _host_prep(x, cos, sin, wq, wk, wv, wo)
    if "nc" not in _NC_CACHE:
        _NC_CACHE["nc"] = _build()
    nc = _NC_CACHE["nc"]
    res = run_bass_kernel_spmd(
        nc, in_maps, core_ids=list(range(N_CORES)), trace=_trace
    )
    out = np.concatenate([res.results[c]["out"] for c in range(N_CORES)], axis=1)
    out = np.ascontiguousarray(out, dtype=np.float32)
    if _trace:
        kernel._last_exec_time_ns = res.exec_time_ns
        kernel._last_result = res
    return out
